# revision 2
# baseline (speedup 1.0000x reference)
"""Trainium2 Bass kernel for the attention-LSTM decoder step.

Contract: kernel(**inputs) takes the FULL (unsharded) numpy inputs and
returns the full output pytree, matching the reference:
    (out (B,1), (h_stack (2,B,H), c_stack (2,B,H)), aw (B,T))

Strategy: pure data parallel over the batch dim across 8 NeuronCores.
Each core processes a 128-row batch shard; weights are replicated.
All heavy compute (attention matmul+softmax, the 256 MiB/core encoder
context reduction, both LSTM cells, prediction head) runs on device.
"""

from contextlib import ExitStack

import numpy as np

import concourse.bass as bass
import concourse.tile as tile
from concourse import bacc, bass_utils, mybir
from concourse.masks import make_identity

dt = mybir.dt
AF = mybir.ActivationFunctionType
ALU = mybir.AluOpType
AX = mybir.AxisListType

B = 1024
H = 1024
T = 512
NUM = 44
IN = 128
G = 4 * H          # 4096, LSTM gate width
N_CORES = 8
P = B // N_CORES   # 128 batch rows per core
TCH = 2            # encoder timesteps per DMA chunk (1 MiB chunks)

_cache: dict = {}


def _build_program(t_steps: int = T):
    """Build the (SPMD, identical-per-core) Bass program."""
    nc = bacc.Bacc("TRN2", target_bir_lowering=False, debug=False,
                   num_devices=N_CORES)
    f32 = dt.float32
    ka = (IN + 2 * H + 127 + 128) // 128      # 18 attn k-tiles (2304 rows)

    enc_d = nc.dram_tensor("enc", [t_steps, P, H], f32, kind="ExternalInput").ap()
    alhs_d = nc.dram_tensor("alhs", [ka, P, P], f32, kind="ExternalInput").ap()
    h01T_d = nc.dram_tensor("h01T", [8, P, P], f32, kind="ExternalInput").ap()
    c00_d = nc.dram_tensor("c00", [P, H], f32, kind="ExternalInput").ap()
    c01_d = nc.dram_tensor("c01", [P, H], f32, kind="ExternalInput").ap()
    attnW_d = nc.dram_tensor("attnW", [ka, P, t_steps], f32, kind="ExternalInput").ap()
    combW_d = nc.dram_tensor("combW", [10, P, IN], f32, kind="ExternalInput").ap()
    W0_d = nc.dram_tensor("W0", [10, P, G], f32, kind="ExternalInput").ap()
    W1_d = nc.dram_tensor("W1", [17, P, G], f32, kind="ExternalInput").ap()
    predw_d = nc.dram_tensor("predw", [P, H], f32, kind="ExternalInput").ap()
    predb_d = nc.dram_tensor("predb", [P, 1], f32, kind="ExternalInput").ap()

    aw_d = nc.dram_tensor("aw", [P, t_steps], f32, kind="ExternalOutput").ap()
    h1_d = nc.dram_tensor("h1", [P, H], f32, kind="ExternalOutput").ap()
    c1_d = nc.dram_tensor("c1", [P, H], f32, kind="ExternalOutput").ap()
    h2_d = nc.dram_tensor("h2", [P, H], f32, kind="ExternalOutput").ap()
    c2_d = nc.dram_tensor("c2", [P, H], f32, kind="ExternalOutput").ap()
    out_d = nc.dram_tensor("out", [P, 1], f32, kind="ExternalOutput").ap()

    n_chunks = t_steps // TCH

    with tile.TileContext(nc) as tc, ExitStack() as ctx:
        const = ctx.enter_context(tc.tile_pool(name="const", bufs=1))
        state = ctx.enter_context(tc.tile_pool(name="state", bufs=1))
        wbig = ctx.enter_context(tc.tile_pool(name="wbig", bufs=2))
        wsm = ctx.enter_context(tc.tile_pool(name="wsm", bufs=3))
        encp = ctx.enter_context(tc.tile_pool(name="encp", bufs=3))
        tmpp = ctx.enter_context(tc.tile_pool(name="tmpp", bufs=3))
        gpre = ctx.enter_context(tc.tile_pool(name="gpre", bufs=5))
        ttp = ctx.enter_context(tc.tile_pool(name="ttp", bufs=2))
        psum = ctx.enter_context(tc.tile_pool(name="psum", bufs=1, space="PSUM"))

        # ---- resident loads -------------------------------------------------
        alhs = const.tile([P, ka, P], f32)
        nc.sync.dma_start(out=alhs, in_=alhs_d.rearrange("k p m -> p k m"))
        h01T = const.tile([P, 8, P], f32)
        nc.sync.dma_start(out=h01T, in_=h01T_d.rearrange("k p m -> p k m"))
        c00 = const.tile([P, H], f32)
        nc.sync.dma_start(out=c00, in_=c00_d)
        c01 = const.tile([P, H], f32)
        nc.sync.dma_start(out=c01, in_=c01_d)
        combW = const.tile([P, 10, IN], f32)
        nc.sync.dma_start(out=combW, in_=combW_d.rearrange("k p m -> p k m"))
        predw = const.tile([P, H], f32)
        nc.sync.dma_start(out=predw, in_=predw_d)
        predb = const.tile([P, 1], f32)
        nc.sync.dma_start(out=predb, in_=predb_d)
        ident = const.tile([P, P], f32)
        make_identity(nc, ident)

        ones_lhsT = alhs[:, ka - 1, :]        # [1;0...] column block

        # ---- attention logits + softmax ------------------------------------
        logits = psum.tile([P, t_steps], f32, tag="ps")
        for k in range(ka):
            wt = wsm.tile([P, t_steps], f32, tag="wa")
            nc.sync.dma_start(out=wt, in_=attnW_d[k])
            nc.tensor.matmul(logits, lhsT=alhs[:, k, :], rhs=wt,
                             start=(k == 0), stop=(k == ka - 1))

        negmax = state.tile([P, 1], f32)
        nc.vector.tensor_reduce(out=negmax, in_=logits, axis=AX.X,
                                op=ALU.max, negate=True)
        aw = state.tile([P, t_steps], f32)
        sumexp = state.tile([P, 1], f32)
        nc.scalar.activation(out=aw, in_=logits, func=AF.Exp,
                             bias=negmax, scale=1.0, accum_out=sumexp)
        rsum = state.tile([P, 1], f32)
        nc.vector.reciprocal(out=rsum, in_=sumexp)
        nc.vector.tensor_scalar_mul(aw, aw, rsum)
        nc.sync.dma_start(out=aw_d, in_=aw)

        # ---- encoder context stream + early LSTM matmuls -------------------
        ctxA = state.tile([P, H], f32)
        ctxB = state.tile([P, H], f32)
        nc.vector.memset(ctxA, 0.0)
        nc.vector.memset(ctxB, 0.0)

        g0p = state.tile([P, G], f32)
        g1p = state.tile([P, G], f32)
        gA = psum.tile([P, G], f32, tag="ps")
        gB = psum.tile([P, G], f32, tag="ps")

        # early weight tasks: (dram tile, lhsT ap, psum tile, start, stop, copy_to)
        wtasks = []
        for k in range(1, 10):
            lh = alhs[:, k, :] if k <= 8 else ones_lhsT
            wtasks.append((W0_d[k], lh, gA, k == 1, k == 9,
                           g0p if k == 9 else None))
        for k in range(8, 17):
            lh = h01T[:, k - 8, :] if k <= 15 else ones_lhsT
            wtasks.append((W1_d[k], lh, gB, k == 8, k == 16,
                           g1p if k == 16 else None))

        wevery = max(1, n_chunks // (len(wtasks) + 1))
        wi = 0
        for ci in range(n_chunks):
            t0 = ci * TCH
            et = encp.tile([P, TCH, H], f32, tag="enc")
            nc.sync.dma_start(out=et,
                              in_=enc_d[t0:t0 + TCH].rearrange("t p h -> p t h"))
            for j in range(TCH):
                t = t0 + j
                tmp = tmpp.tile([P, H], f32, tag="tmp")
                nc.scalar.activation(out=tmp, in_=et[:, j, :], func=AF.Copy,
                                     scale=aw[:, t:t + 1])
                acc = ctxA if t % 2 == 0 else ctxB
                nc.vector.tensor_add(acc, acc, tmp)
            if wi < len(wtasks) and (ci + 1) % wevery == 0:
                wdram, lh, gps, st, sp, cpy = wtasks[wi]
                wt = wbig.tile([P, G], f32, tag="wb")
                nc.sync.dma_start(out=wt, in_=wdram)
                for n in range(8):
                    ns = slice(n * 512, (n + 1) * 512)
                    nc.tensor.matmul(gps[:, ns], lhsT=lh, rhs=wt[:, ns],
                                     start=st, stop=sp)
                if cpy is not None:
                    nc.scalar.copy(cpy, gps)
                wi += 1
        while wi < len(wtasks):  # leftovers (shouldn't happen for T=512)
            wdram, lh, gps, st, sp, cpy = wtasks[wi]
            wt = wbig.tile([P, G], f32, tag="wb")
            nc.sync.dma_start(out=wt, in_=wdram)
            for n in range(8):
                ns = slice(n * 512, (n + 1) * 512)
                nc.tensor.matmul(gps[:, ns], lhsT=lh, rhs=wt[:, ns],
                                 start=st, stop=sp)
            if cpy is not None:
                nc.scalar.copy(cpy, gps)
            wi += 1

        nc.vector.tensor_add(ctxA, ctxA, ctxB)   # final ctx in ctxA

        # ---- ctx^T, comb, comb^T -------------------------------------------
        ctxT = state.tile([P, 8, P], f32)
        tp = psum.tile([P, G], f32, tag="ps")
        for i in range(8):
            nc.tensor.transpose(tp[:, i * 512:i * 512 + P],
                                ctxA[:, i * P:(i + 1) * P], ident)
            nc.scalar.copy(ctxT[:, i, :], tp[:, i * 512:i * 512 + P])

        cps = psum.tile([P, IN], f32, tag="ps")
        for ki in range(10):
            if ki == 0:
                lh = alhs[:, 0, :]
            elif ki <= 8:
                lh = ctxT[:, ki - 1, :]
            else:
                lh = ones_lhsT
            nc.tensor.matmul(cps, lhsT=lh, rhs=combW[:, ki, :],
                             start=(ki == 0), stop=(ki == 9))
        comb = state.tile([P, IN], f32)
        nc.scalar.copy(comb, cps)
        tp2 = psum.tile([P, P], f32, tag="ps")
        nc.tensor.transpose(tp2, comb, ident)
        combT = state.tile([P, P], f32)
        nc.scalar.copy(combT, tp2)

        # ---- LSTM cell helper ----------------------------------------------
        def lstm_gates(gps, gpart, c_in, h_out, c_out, h_dram, c_dram):
            pre = []
            for q in range(4):
                qs = slice(q * H, (q + 1) * H)
                pq = gpre.tile([P, H], f32, tag="g")
                nc.vector.tensor_add(pq, gps[:, qs], gpart[:, qs])
                nc.scalar.activation(pq, pq,
                                     AF.Tanh if q == 2 else AF.Sigmoid)
                pre.append(pq)
            t1 = ttp.tile([P, H], f32, tag="t")
            nc.vector.tensor_mul(t1, pre[1], c_in)         # sig(f)*c
            t2 = ttp.tile([P, H], f32, tag="t")
            nc.vector.tensor_mul(t2, pre[0], pre[2])       # sig(i)*tanh(g)
            nc.vector.tensor_add(c_out, t1, t2)
            tcn = gpre.tile([P, H], f32, tag="g")
            nc.scalar.activation(tcn, c_out, AF.Tanh)
            nc.vector.tensor_mul(h_out, pre[3], tcn)       # sig(o)*tanh(c')
            nc.sync.dma_start(out=c_dram, in_=c_out)
            nc.sync.dma_start(out=h_dram, in_=h_out)

        # ---- LSTM cell 0 ----------------------------------------------------
        g0ps = psum.tile([P, G], f32, tag="ps")
        w0t0 = wbig.tile([P, G], f32, tag="wb")
        nc.sync.dma_start(out=w0t0, in_=W0_d[0])
        for n in range(8):
            ns = slice(n * 512, (n + 1) * 512)
            nc.tensor.matmul(g0ps[:, ns], lhsT=combT, rhs=w0t0[:, ns],
                             start=True, stop=True)
        h1 = state.tile([P, H], f32)
        c1 = state.tile([P, H], f32)
        lstm_gates(g0ps, g0p, c00, h1, c1, h1_d, c1_d)

        # ---- h1^T -----------------------------------------------------------
        h1T = state.tile([P, 8, P], f32)
        tp3 = psum.tile([P, G], f32, tag="ps")
        for i in range(8):
            nc.tensor.transpose(tp3[:, i * 512:i * 512 + P],
                                h1[:, i * P:(i + 1) * P], ident)
            nc.scalar.copy(h1T[:, i, :], tp3[:, i * 512:i * 512 + P])

        # ---- LSTM cell 1 ----------------------------------------------------
        g1ps = psum.tile([P, G], f32, tag="ps")
        for k in range(8):
            wt = wbig.tile([P, G], f32, tag="wb")
            nc.sync.dma_start(out=wt, in_=W1_d[k])
            for n in range(8):
                ns = slice(n * 512, (n + 1) * 512)
                nc.tensor.matmul(g1ps[:, ns], lhsT=h1T[:, k, :], rhs=wt[:, ns],
                                 start=(k == 0), stop=(k == 7))
        h2 = state.tile([P, H], f32)
        c2 = state.tile([P, H], f32)
        lstm_gates(g1ps, g1p, c01, h2, c2, h2_d, c2_d)

        # ---- prediction head ------------------------------------------------
        junk = tmpp.tile([P, H], f32, tag="tmp")
        nc.vector.tensor_mul(junk, h2, predw)
        psb = state.tile([P, 1], f32)
        nc.vector.tensor_reduce(out=psb, in_=junk, axis=AX.X, op=ALU.add)
        out_sb = state.tile([P, 1], f32)
        nc.vector.tensor_add(out_sb, psb, predb)
        nc.sync.dma_start(out=out_d, in_=out_sb)

    nc.compile()
    return nc


def _host_prep(x, x_emb, h0, c0, encoder_outputs, emb0, emb1, emb2, emb3,
               attn_w, attn_b, comb_w, comb_b,
               w_ih0, w_hh0, b_ih0, b_hh0, w_ih1, w_hh1, b_ih1, b_hh1,
               pred_w, pred_b):
    """Shard + lay out inputs for the device program. Returns in_maps."""
    f32 = np.float32
    t_steps = encoder_outputs.shape[0]
    x = np.asarray(x, f32)
    ids = np.asarray(x_emb).astype(np.int64)
    h0 = np.asarray(h0, f32)
    c0 = np.asarray(c0, f32)
    enc = np.asarray(encoder_outputs, f32)

    xt = x[:, 0, :]
    e = np.concatenate([np.asarray(emb0, f32)[ids[:, 0, 0]],
                        np.asarray(emb1, f32)[ids[:, 0, 1]],
                        np.asarray(emb2, f32)[ids[:, 0, 2]],
                        np.asarray(emb3, f32)[ids[:, 0, 3]]], axis=1)
    x_rnn = np.concatenate([xt, e], axis=1)                    # (B, IN)

    def padrows(m, bias_row, tot):
        pad = np.zeros((tot - m.shape[0] - 1, m.shape[1]), f32)
        return np.concatenate([m, bias_row[None, :].astype(f32), pad], 0)

    ka = 18
    attn_W = padrows(np.asarray(attn_w, f32).T, np.asarray(attn_b, f32),
                     ka * 128).reshape(ka, 128, t_steps)
    comb_W = padrows(np.asarray(comb_w, f32).T, np.asarray(comb_b, f32),
                     10 * 128).reshape(10, 128, IN)
    W0 = padrows(np.concatenate([np.asarray(w_ih0, f32).T,
                                 np.asarray(w_hh0, f32).T], 0),
                 np.asarray(b_ih0, f32) + np.asarray(b_hh0, f32),
                 10 * 128).reshape(10, 128, G)
    W1 = padrows(np.concatenate([np.asarray(w_ih1, f32).T,
                                 np.asarray(w_hh1, f32).T], 0),
                 np.asarray(b_ih1, f32) + np.asarray(b_hh1, f32),
                 17 * 128).reshape(17, 128, G)
    predw_b = np.ascontiguousarray(
        np.broadcast_to(np.asarray(pred_w, f32).reshape(1, H), (P, H)))
    predb_b = np.full((P, 1), np.asarray(pred_b, f32).reshape(()), f32)

    in_maps = []
    for ci in range(N_CORES):
        r = slice(ci * P, (ci + 1) * P)
        attn_inT = np.concatenate([x_rnn[r], h0[0][r], c0[0][r]], 1).T
        alhs = np.concatenate(
            [attn_inT, np.ones((1, P), f32),
             np.zeros((ka * 128 - attn_inT.shape[0] - 1, P), f32)],
            0).reshape(ka, 128, P)
        in_maps.append({
            "enc": np.ascontiguousarray(enc[:, r, :]),
            "alhs": np.ascontiguousarray(alhs),
            "h01T": np.ascontiguousarray(h0[1][r].T).reshape(8, 128, P),
            "c00": np.ascontiguousarray(c0[0][r]),
            "c01": np.ascontiguousarray(c0[1][r]),
            "attnW": attn_W, "combW": comb_W, "W0": W0, "W1": W1,
            "predw": predw_b, "predb": predb_b,
        })
    return in_maps


def _assemble(results):
    """Gather per-core output dicts back into the full reference pytree."""

    def cat(name):
        return np.concatenate([results[i][name] for i in range(N_CORES)], 0)

    out = cat("out")
    h_stack = np.stack([cat("h1"), cat("h2")])
    c_stack = np.stack([cat("c1"), cat("c2")])
    aw = cat("aw")
    return out, (h_stack, c_stack), aw


def kernel(**inputs):
    t_steps = np.asarray(inputs["encoder_outputs"]).shape[0]
    key = ("prog", t_steps)
    if key not in _cache:
        _cache[key] = _build_program(t_steps)
    nc = _cache[key]
    in_maps = _host_prep(**inputs)
    res = bass_utils.run_bass_kernel_spmd(
        nc, in_maps, core_ids=list(range(N_CORES)))
    return _assemble(res.results)


# revision 3
# speedup vs baseline: 1.2450x; 1.2450x over previous
"""Trainium2 Bass kernel for the attention-LSTM decoder step.

Contract: kernel(**inputs) takes the FULL (unsharded) numpy inputs and
returns the full output pytree, matching the reference:
    (out (B,1), (h_stack (2,B,H), c_stack (2,B,H)), aw (B,T))

Strategy: pure data parallel over the batch dim across 8 NeuronCores.
Each core processes a 128-row batch shard; weights are replicated.
All heavy compute (attention matmul+softmax, the 256 MiB/core encoder
context reduction, both LSTM cells, prediction head) runs on device.
"""

from contextlib import ExitStack

import numpy as np

import concourse.bass as bass
import concourse.tile as tile
from concourse import bacc, bass_utils, mybir
from concourse.masks import make_identity

dt = mybir.dt
AF = mybir.ActivationFunctionType
ALU = mybir.AluOpType
AX = mybir.AxisListType

B = 1024
H = 1024
T = 512
NUM = 44
IN = 128
G = 4 * H          # 4096, LSTM gate width
N_CORES = 8
P = B // N_CORES   # 128 batch rows per core
TCH = 2            # encoder timesteps per DMA chunk (1 MiB chunks)

_cache: dict = {}


def _build_program(t_steps: int = T):
    """Build the (SPMD, identical-per-core) Bass program."""
    nc = bacc.Bacc("TRN2", target_bir_lowering=False, debug=False,
                   num_devices=N_CORES)
    f32 = dt.float32
    ka = (IN + 2 * H + 127 + 128) // 128      # 18 attn k-tiles (2304 rows)

    enc_d = nc.dram_tensor("enc", [t_steps, P, H], f32, kind="ExternalInput").ap()
    alhs_d = nc.dram_tensor("alhs", [ka, P, P], f32, kind="ExternalInput").ap()
    f16 = dt.float16
    h01T_d = nc.dram_tensor("h01T", [8, P, P], f16, kind="ExternalInput").ap()
    h00h_d = nc.dram_tensor("h00h", [8, P, P], f16, kind="ExternalInput").ap()
    onesh_d = nc.dram_tensor("onesh", [P, P], f16, kind="ExternalInput").ap()
    c00_d = nc.dram_tensor("c00", [P, H], f32, kind="ExternalInput").ap()
    c01_d = nc.dram_tensor("c01", [P, H], f32, kind="ExternalInput").ap()
    attnW_d = nc.dram_tensor("attnW", [ka, P, t_steps], f32, kind="ExternalInput").ap()
    combW_d = nc.dram_tensor("combW", [10, P, IN], f32, kind="ExternalInput").ap()
    W0_d = nc.dram_tensor("W0", [10, P, G], f16, kind="ExternalInput").ap()
    W1_d = nc.dram_tensor("W1", [17, P, G], f16, kind="ExternalInput").ap()
    predw_d = nc.dram_tensor("predw", [P, H], f32, kind="ExternalInput").ap()
    predb_d = nc.dram_tensor("predb", [P, 1], f32, kind="ExternalInput").ap()

    aw_d = nc.dram_tensor("aw", [P, t_steps], f32, kind="ExternalOutput").ap()
    h1_d = nc.dram_tensor("h1", [P, H], f32, kind="ExternalOutput").ap()
    c1_d = nc.dram_tensor("c1", [P, H], f32, kind="ExternalOutput").ap()
    h2_d = nc.dram_tensor("h2", [P, H], f32, kind="ExternalOutput").ap()
    c2_d = nc.dram_tensor("c2", [P, H], f32, kind="ExternalOutput").ap()
    out_d = nc.dram_tensor("out", [P, 1], f32, kind="ExternalOutput").ap()

    n_chunks = t_steps // TCH

    with tile.TileContext(nc) as tc, ExitStack() as ctx:
        const = ctx.enter_context(tc.tile_pool(name="const", bufs=1))
        state = ctx.enter_context(tc.tile_pool(name="state", bufs=1))
        wbig = ctx.enter_context(tc.tile_pool(name="wbig", bufs=2))
        wsm = ctx.enter_context(tc.tile_pool(name="wsm", bufs=3))
        encp = ctx.enter_context(tc.tile_pool(name="encp", bufs=3))
        tmpp = ctx.enter_context(tc.tile_pool(name="tmpp", bufs=3))
        gpre = ctx.enter_context(tc.tile_pool(name="gpre", bufs=5))
        ttp = ctx.enter_context(tc.tile_pool(name="ttp", bufs=2))
        psum = ctx.enter_context(tc.tile_pool(name="psum", bufs=1, space="PSUM"))

        # ---- resident loads -------------------------------------------------
        alhs = const.tile([P, ka, P], f32)
        nc.sync.dma_start(out=alhs, in_=alhs_d.rearrange("k p m -> p k m"))
        h01T = const.tile([P, 8, P], f16)
        nc.sync.dma_start(out=h01T, in_=h01T_d.rearrange("k p m -> p k m"))
        h00h = const.tile([P, 8, P], f16)
        nc.sync.dma_start(out=h00h, in_=h00h_d.rearrange("k p m -> p k m"))
        onesh = const.tile([P, P], f16)
        nc.sync.dma_start(out=onesh, in_=onesh_d)
        c00 = const.tile([P, H], f32)
        nc.sync.dma_start(out=c00, in_=c00_d)
        c01 = const.tile([P, H], f32)
        nc.sync.dma_start(out=c01, in_=c01_d)
        combW = const.tile([P, 10, IN], f32)
        nc.sync.dma_start(out=combW, in_=combW_d.rearrange("k p m -> p k m"))
        predw = const.tile([P, H], f32)
        nc.sync.dma_start(out=predw, in_=predw_d)
        predb = const.tile([P, 1], f32)
        nc.sync.dma_start(out=predb, in_=predb_d)
        ident = const.tile([P, P], f32)
        make_identity(nc, ident)

        ones_lhsT = alhs[:, ka - 1, :]        # [1;0...] column block

        # ---- attention logits + softmax ------------------------------------
        logits = psum.tile([P, t_steps], f32, tag="ps")
        for k in range(ka):
            wt = wsm.tile([P, t_steps], f32, tag="wa")
            nc.sync.dma_start(out=wt, in_=attnW_d[k])
            nc.tensor.matmul(logits, lhsT=alhs[:, k, :], rhs=wt,
                             start=(k == 0), stop=(k == ka - 1))

        negmax = state.tile([P, 1], f32)
        nc.vector.tensor_reduce(out=negmax, in_=logits, axis=AX.X,
                                op=ALU.max, negate=True)
        aw = state.tile([P, t_steps], f32)
        sumexp = state.tile([P, 1], f32)
        nc.scalar.activation(out=aw, in_=logits, func=AF.Exp,
                             bias=negmax, scale=1.0, accum_out=sumexp)
        rsum = state.tile([P, 1], f32)
        nc.vector.reciprocal(out=rsum, in_=sumexp)
        nc.vector.tensor_scalar_mul(aw, aw, rsum)
        nc.sync.dma_start(out=aw_d, in_=aw)

        # ---- encoder context stream + early LSTM matmuls -------------------
        ctxA = state.tile([P, H], f32)
        ctxB = state.tile([P, H], f32)
        nc.vector.memset(ctxA, 0.0)
        nc.vector.memset(ctxB, 0.0)

        g0p = state.tile([P, G], f32)
        g1p = state.tile([P, G], f32)
        gA = psum.tile([P, G], f32, tag="ps")
        gB = psum.tile([P, G], f32, tag="ps")

        # early weight tasks: (dram tile, lhsT ap, psum tile, start, stop, copy_to)
        wtasks = []
        for k in range(1, 10):
            lh = h00h[:, k - 1, :] if k <= 8 else onesh
            wtasks.append((W0_d[k], lh, gA, k == 1, k == 9,
                           g0p if k == 9 else None))
        for k in range(8, 17):
            lh = h01T[:, k - 8, :] if k <= 15 else onesh
            wtasks.append((W1_d[k], lh, gB, k == 8, k == 16,
                           g1p if k == 16 else None))

        wevery = max(1, n_chunks // (len(wtasks) + 1))
        wi = 0
        for ci in range(n_chunks):
            t0 = ci * TCH
            et = encp.tile([P, TCH, H], f32, tag="enc")
            nc.sync.dma_start(out=et,
                              in_=enc_d[t0:t0 + TCH].rearrange("t p h -> p t h"))
            for j in range(TCH):
                t = t0 + j
                tmp = tmpp.tile([P, H], f32, tag="tmp")
                nc.scalar.activation(out=tmp, in_=et[:, j, :], func=AF.Copy,
                                     scale=aw[:, t:t + 1])
                acc = ctxA if t % 2 == 0 else ctxB
                nc.vector.tensor_add(acc, acc, tmp)
            if wi < len(wtasks) and (ci + 1) % wevery == 0:
                wdram, lh, gps, st, sp, cpy = wtasks[wi]
                wt = wbig.tile([P, G], f16, tag="wb")
                nc.sync.dma_start(out=wt, in_=wdram)
                for n in range(8):
                    ns = slice(n * 512, (n + 1) * 512)
                    nc.tensor.matmul(gps[:, ns], lhsT=lh, rhs=wt[:, ns],
                                     start=st, stop=sp)
                if cpy is not None:
                    nc.scalar.copy(cpy, gps)
                wi += 1
        while wi < len(wtasks):  # leftovers (shouldn't happen for T=512)
            wdram, lh, gps, st, sp, cpy = wtasks[wi]
            wt = wbig.tile([P, G], f16, tag="wb")
            nc.sync.dma_start(out=wt, in_=wdram)
            for n in range(8):
                ns = slice(n * 512, (n + 1) * 512)
                nc.tensor.matmul(gps[:, ns], lhsT=lh, rhs=wt[:, ns],
                                 start=st, stop=sp)
            if cpy is not None:
                nc.scalar.copy(cpy, gps)
            wi += 1

        nc.vector.tensor_add(ctxA, ctxA, ctxB)   # final ctx in ctxA

        # ---- ctx^T, comb, comb^T -------------------------------------------
        ctxT = state.tile([P, 8, P], f32)
        tp = psum.tile([P, G], f32, tag="ps")
        for i in range(8):
            nc.tensor.transpose(tp[:, i * 512:i * 512 + P],
                                ctxA[:, i * P:(i + 1) * P], ident)
            nc.scalar.copy(ctxT[:, i, :], tp[:, i * 512:i * 512 + P])

        cps = psum.tile([P, IN], f32, tag="ps")
        for ki in range(10):
            if ki == 0:
                lh = alhs[:, 0, :]
            elif ki <= 8:
                lh = ctxT[:, ki - 1, :]
            else:
                lh = ones_lhsT
            nc.tensor.matmul(cps, lhsT=lh, rhs=combW[:, ki, :],
                             start=(ki == 0), stop=(ki == 9))
        comb = state.tile([P, IN], f32)
        nc.scalar.copy(comb, cps)
        tp2 = psum.tile([P, P], f32, tag="ps")
        nc.tensor.transpose(tp2, comb, ident)
        combT = state.tile([P, P], f16)
        nc.scalar.copy(combT, tp2)

        # ---- LSTM cell helper ----------------------------------------------
        def lstm_gates(gps, gpart, c_in, h_out, c_out, h_dram, c_dram):
            pre = []
            for q in range(4):
                qs = slice(q * H, (q + 1) * H)
                pq = gpre.tile([P, H], f32, tag="g")
                nc.vector.tensor_add(pq, gps[:, qs], gpart[:, qs])
                nc.scalar.activation(pq, pq,
                                     AF.Tanh if q == 2 else AF.Sigmoid)
                pre.append(pq)
            t1 = ttp.tile([P, H], f32, tag="t")
            nc.vector.tensor_mul(t1, pre[1], c_in)         # sig(f)*c
            t2 = ttp.tile([P, H], f32, tag="t")
            nc.vector.tensor_mul(t2, pre[0], pre[2])       # sig(i)*tanh(g)
            nc.vector.tensor_add(c_out, t1, t2)
            tcn = gpre.tile([P, H], f32, tag="g")
            nc.scalar.activation(tcn, c_out, AF.Tanh)
            nc.vector.tensor_mul(h_out, pre[3], tcn)       # sig(o)*tanh(c')
            nc.sync.dma_start(out=c_dram, in_=c_out)
            nc.sync.dma_start(out=h_dram, in_=h_out)

        # ---- LSTM cell 0 ----------------------------------------------------
        g0ps = psum.tile([P, G], f32, tag="ps")
        w0t0 = wbig.tile([P, G], f16, tag="wb")
        nc.sync.dma_start(out=w0t0, in_=W0_d[0])
        for n in range(8):
            ns = slice(n * 512, (n + 1) * 512)
            nc.tensor.matmul(g0ps[:, ns], lhsT=combT, rhs=w0t0[:, ns],
                             start=True, stop=True)
        h1 = state.tile([P, H], f32)
        c1 = state.tile([P, H], f32)
        lstm_gates(g0ps, g0p, c00, h1, c1, h1_d, c1_d)

        # ---- h1^T -----------------------------------------------------------
        h1T = state.tile([P, 8, P], f16)
        tp3 = psum.tile([P, G], f32, tag="ps")
        for i in range(8):
            nc.tensor.transpose(tp3[:, i * 512:i * 512 + P],
                                h1[:, i * P:(i + 1) * P], ident)
            nc.scalar.copy(h1T[:, i, :], tp3[:, i * 512:i * 512 + P])

        # ---- LSTM cell 1 ----------------------------------------------------
        g1ps = psum.tile([P, G], f32, tag="ps")
        for k in range(8):
            wt = wbig.tile([P, G], f16, tag="wb")
            nc.sync.dma_start(out=wt, in_=W1_d[k])
            for n in range(8):
                ns = slice(n * 512, (n + 1) * 512)
                nc.tensor.matmul(g1ps[:, ns], lhsT=h1T[:, k, :], rhs=wt[:, ns],
                                 start=(k == 0), stop=(k == 7))
        h2 = state.tile([P, H], f32)
        c2 = state.tile([P, H], f32)
        lstm_gates(g1ps, g1p, c01, h2, c2, h2_d, c2_d)

        # ---- prediction head ------------------------------------------------
        junk = tmpp.tile([P, H], f32, tag="tmp")
        nc.vector.tensor_mul(junk, h2, predw)
        psb = state.tile([P, 1], f32)
        nc.vector.tensor_reduce(out=psb, in_=junk, axis=AX.X, op=ALU.add)
        out_sb = state.tile([P, 1], f32)
        nc.vector.tensor_add(out_sb, psb, predb)
        nc.sync.dma_start(out=out_d, in_=out_sb)

    nc.compile()
    return nc


def _host_prep(x, x_emb, h0, c0, encoder_outputs, emb0, emb1, emb2, emb3,
               attn_w, attn_b, comb_w, comb_b,
               w_ih0, w_hh0, b_ih0, b_hh0, w_ih1, w_hh1, b_ih1, b_hh1,
               pred_w, pred_b):
    """Shard + lay out inputs for the device program. Returns in_maps."""
    f32 = np.float32
    t_steps = encoder_outputs.shape[0]
    x = np.asarray(x, f32)
    ids = np.asarray(x_emb).astype(np.int64)
    h0 = np.asarray(h0, f32)
    c0 = np.asarray(c0, f32)
    enc = np.asarray(encoder_outputs, f32)

    xt = x[:, 0, :]
    e = np.concatenate([np.asarray(emb0, f32)[ids[:, 0, 0]],
                        np.asarray(emb1, f32)[ids[:, 0, 1]],
                        np.asarray(emb2, f32)[ids[:, 0, 2]],
                        np.asarray(emb3, f32)[ids[:, 0, 3]]], axis=1)
    x_rnn = np.concatenate([xt, e], axis=1)                    # (B, IN)

    def padrows(m, bias_row, tot):
        pad = np.zeros((tot - m.shape[0] - 1, m.shape[1]), f32)
        return np.concatenate([m, bias_row[None, :].astype(f32), pad], 0)

    ka = 18
    attn_W = padrows(np.asarray(attn_w, f32).T, np.asarray(attn_b, f32),
                     ka * 128).reshape(ka, 128, t_steps)
    comb_W = padrows(np.asarray(comb_w, f32).T, np.asarray(comb_b, f32),
                     10 * 128).reshape(10, 128, IN)
    W0 = padrows(np.concatenate([np.asarray(w_ih0, f32).T,
                                 np.asarray(w_hh0, f32).T], 0),
                 np.asarray(b_ih0, f32) + np.asarray(b_hh0, f32),
                 10 * 128).reshape(10, 128, G).astype(np.float16)
    W1 = padrows(np.concatenate([np.asarray(w_ih1, f32).T,
                                 np.asarray(w_hh1, f32).T], 0),
                 np.asarray(b_ih1, f32) + np.asarray(b_hh1, f32),
                 17 * 128).reshape(17, 128, G).astype(np.float16)
    onesh = np.zeros((128, P), np.float16)
    onesh[0, :] = 1.0
    predw_b = np.ascontiguousarray(
        np.broadcast_to(np.asarray(pred_w, f32).reshape(1, H), (P, H)))
    predb_b = np.full((P, 1), np.asarray(pred_b, f32).reshape(()), f32)

    in_maps = []
    for ci in range(N_CORES):
        r = slice(ci * P, (ci + 1) * P)
        attn_inT = np.concatenate([x_rnn[r], h0[0][r], c0[0][r]], 1).T
        alhs = np.concatenate(
            [attn_inT, np.ones((1, P), f32),
             np.zeros((ka * 128 - attn_inT.shape[0] - 1, P), f32)],
            0).reshape(ka, 128, P)
        in_maps.append({
            "enc": np.ascontiguousarray(enc[:, r, :]),
            "alhs": np.ascontiguousarray(alhs),
            "h01T": np.ascontiguousarray(
                h0[1][r].T.astype(np.float16)).reshape(8, 128, P),
            "h00h": np.ascontiguousarray(
                h0[0][r].T.astype(np.float16)).reshape(8, 128, P),
            "onesh": onesh,
            "c00": np.ascontiguousarray(c0[0][r]),
            "c01": np.ascontiguousarray(c0[1][r]),
            "attnW": attn_W, "combW": comb_W, "W0": W0, "W1": W1,
            "predw": predw_b, "predb": predb_b,
        })
    return in_maps


def _assemble(results):
    """Gather per-core output dicts back into the full reference pytree."""

    def cat(name):
        return np.concatenate([results[i][name] for i in range(N_CORES)], 0)

    out = cat("out")
    h_stack = np.stack([cat("h1"), cat("h2")])
    c_stack = np.stack([cat("c1"), cat("c2")])
    aw = cat("aw")
    return out, (h_stack, c_stack), aw


def kernel(**inputs):
    t_steps = np.asarray(inputs["encoder_outputs"]).shape[0]
    key = ("prog", t_steps)
    if key not in _cache:
        _cache[key] = _build_program(t_steps)
    nc = _cache[key]
    in_maps = _host_prep(**inputs)
    res = bass_utils.run_bass_kernel_spmd(
        nc, in_maps, core_ids=list(range(N_CORES)))
    return _assemble(res.results)
